# revision 22
# baseline (speedup 1.0000x reference)
"""Fused multi-head cross-attention with relation branch, sharded over 8 NeuronCores.

Sharding: data-parallel over batch (4) x tensor-parallel over head halves (2).
Core c handles batch c//2, heads [8*(c%2), 8*(c%2)+8). Each core computes its
partial output projection; the host sums the two partials per batch and adds bo.

Device data flow (per core), v2:
  - q/k/rk projections emitted transposed: qT/kT/rkT [512 local dims, L]
    (4 chunks of 128 dims = head pairs (2dc, 2dc+1) at partitions 0-63/64-127)
  - v/rv projections natural: per lk-chunk [128 lk, 8 heads x 64 dims].
  - scores sT[lk, lq] = kT.T @ qT per head; the two heads of a dim chunk run
    as one row-tiled concurrent pair (K=64 at array rows 0-63 / 64-127).
  - exp + mask + 1/sqrt(dk) fused into one ACT op per score tile.
  - PV: col-tiled concurrent pair per branch: head a -> psum rows 0:64
    (tile_position (0,0)), head b -> rows 64:128 ((0,64)); so xv/xr psum
    tiles land directly in the [2-head dims, lq] layout xf needs.
  - softmax denominators: 4-way col-tiled M=32 matmuls (all-ones weights)
    accumulate sum(p) into one psum tile (rows 0:32 = vis head a, 32:64 =
    vis head b, 64:96 = rel a, 96:128 = rel b); DVE reciprocal -> SBUF;
    gpsimd partition_broadcast expands each group row to 64 partitions;
    DVE combines xf = xv*rv + xr*rr. No DRAM round-trips.
  - output projection per lqh: 8 psum accumulators over 4 dim chunks,
    copies emitted as bf16, yT shipped bf16 (host sums partials in f32).
  - ~8 warmup matmuls on a memset tile at t~6.5us un-throttle the PE HAM
    clock gate before real data arrives.
"""

import math

import numpy as np

B, LQ, LK, D, H = 4, 1024, 1024, 1024, 16
DK = D // H
SCALE = 1.0 / math.sqrt(DK)
N_CORES = 8
HD = D // 2  # local dims per core (8 heads * 64)
# Keys are compacted host-side: only unmasked keys are shipped (padded to LKP
# with dummy rows whose mask bias is -1e9, so exp()=0 -> exact same math).
LKP = 640

DEBUG_DUMP = False

_CACHE = {}


def _build_program(lkp=LKP):
    import concourse.bacc as bacc
    import concourse.mybir as mybir
    import concourse.tile as tile

    LKP = lkp
    NM = LKP // 128

    f32 = mybir.dt.float32
    bf16 = mybir.dt.bfloat16
    Exp = mybir.ActivationFunctionType.Exp
    Add = mybir.AluOpType.add
    Mult = mybir.AluOpType.mult

    nc = bacc.Bacc(
        "TRN2",
        target_bir_lowering=False,
        debug=False,
        enable_asserts=False,
        num_devices=N_CORES,
    )

    # DRAM I/O (per-core shapes; host shards/pre-transposes/casts).
    xqT = nc.dram_tensor("xqT", [D, LQ], bf16, kind="ExternalInput").ap()
    xkT = nc.dram_tensor("xkT", [D, LKP], bf16, kind="ExternalInput").ap()
    xrT = nc.dram_tensor("xrT", [D, LKP], bf16, kind="ExternalInput").ap()
    xvT = nc.dram_tensor("xvT", [D, LKP], bf16, kind="ExternalInput").ap()
    wqT = nc.dram_tensor("wqT", [D, HD], bf16, kind="ExternalInput").ap()
    wkT = nc.dram_tensor("wkT", [D, HD], bf16, kind="ExternalInput").ap()
    wrkT = nc.dram_tensor("wrkT", [D, HD], bf16, kind="ExternalInput").ap()
    wvT = nc.dram_tensor("wvT", [D, HD], bf16, kind="ExternalInput").ap()
    wrvT = nc.dram_tensor("wrvT", [D, HD], bf16, kind="ExternalInput").ap()
    woT = nc.dram_tensor("woT", [HD, D], bf16, kind="ExternalInput").ap()
    bq_pc = nc.dram_tensor("bq_pc", [128, 4], f32, kind="ExternalInput").ap()
    bk_pc = nc.dram_tensor("bk_pc", [128, 4], f32, kind="ExternalInput").ap()
    brk_pc = nc.dram_tensor("brk_pc", [128, 4], f32, kind="ExternalInput").ap()
    bv_bc = nc.dram_tensor("bv_bc", [128, HD], f32, kind="ExternalInput").ap()
    brv_bc = nc.dram_tensor("brv_bc", [128, HD], f32, kind="ExternalInput").ap()
    maskb = nc.dram_tensor("maskb", [128, NM], f32, kind="ExternalInput").ap()
    yT = nc.dram_tensor("yT", [D, LQ], bf16, kind="ExternalOutput").ap()
    if DEBUG_DUMP:
        dbg_den = nc.dram_tensor("dbg_den", [128, 512], f32, kind="ExternalOutput").ap()
        dbg_rec = nc.dram_tensor("dbg_rec", [128, 512], f32, kind="ExternalOutput").ap()
        dbg_bcv = nc.dram_tensor("dbg_bcv", [128, 512], f32, kind="ExternalOutput").ap()
        dbg_xv = nc.dram_tensor("dbg_xv", [128, 512], f32, kind="ExternalOutput").ap()

    with tile.TileContext(nc) as tc:
        from contextlib import ExitStack

        with ExitStack() as ctx:
            # Persistent SBUF tensors.
            persist = ctx.enter_context(tc.tile_pool(name="persist", bufs=1))
            qT_sb = persist.tile([128, 4 * LQ], bf16, tag="qT")
            kT_sb = persist.tile([128, 4 * LKP], bf16, tag="kT")
            rkT_sb = persist.tile([128, 4 * LKP], bf16, tag="rkT")
            v_sb = persist.tile([128, NM * 8 * 64], bf16, tag="v")
            rv_sb = persist.tile([128, NM * 8 * 64], bf16, tag="rv")
            xf_sb = persist.tile([128, 4 * LQ], bf16, tag="xf")
            wo_sb = persist.tile([128, 4 * LQ], bf16, tag="wo")
            maskb_sb = persist.tile([128, NM], f32, tag="maskb")
            bq_sb = persist.tile([128, 4], f32, tag="bq")
            bk_sb = persist.tile([128, 4], f32, tag="bk")
            brk_sb = persist.tile([128, 4], f32, tag="brk")
            bv_sb = persist.tile([128, HD], f32, tag="bv")
            brv_sb = persist.tile([128, HD], f32, tag="brv")
            ones_bf = persist.tile([128, 64], bf16, tag="onesb")
            warm_sb = persist.tile([128, 512], bf16, tag="warm")

            # Memsets first: warmup matmuls depend only on these.
            nc.vector.memset(warm_sb[:], 0.125)
            nc.vector.memset(ones_bf[:], 1.0)

            # Small parameter DMAs on the gpsimd queue.
            nc.gpsimd.dma_start(out=maskb_sb[:], in_=maskb)
            nc.gpsimd.dma_start(out=bq_sb[:], in_=bq_pc)
            nc.gpsimd.dma_start(out=bk_sb[:], in_=bk_pc)
            nc.gpsimd.dma_start(out=brk_sb[:], in_=brk_pc)
            nc.gpsimd.dma_start(out=bv_sb[:], in_=bv_bc)
            nc.gpsimd.dma_start(out=brv_sb[:], in_=brv_bc)

            v4 = v_sb[:].rearrange("p (m h c) -> p m h c", m=NM, h=8, c=64)
            rv4 = rv_sb[:].rearrange("p (m h c) -> p m h c", m=NM, h=8, c=64)

            # Score/exp pools opened BEFORE the projection pools so their PSUM
            # banks are disjoint from the projection psum banks.
            spool = ctx.enter_context(tc.tile_pool(name="spool", bufs=2, space="PSUM"))
            ppool = ctx.enter_context(tc.tile_pool(name="ppool", bufs=18))

            p_tiles = {}

            def emit_scores(lqh):
                for dc in range(4):
                    qsl = slice(1024 * dc + 512 * lqh, 1024 * dc + 512 * lqh + 512)
                    for m in range(NM):
                        ksl = slice(LKP * dc + 128 * m, LKP * dc + 128 * m + 128)
                        for br, kt in ((0, kT_sb), (1, rkT_sb)):
                            s = spool.tile([128, 1024], f32, tag="spool", name="s")
                            nc.tensor.matmul(
                                s[:, 0:512], kt[0:64, ksl], qT_sb[0:64, qsl]
                            )
                            nc.tensor.matmul(
                                s[:, 512:1024], kt[64:128, ksl], qT_sb[64:128, qsl]
                            )
                            p = ppool.tile([128, 1024], bf16, tag="ppool", name="p")
                            nc.scalar.activation(
                                p[:],
                                s[:],
                                Exp,
                                bias=maskb_sb[:, m : m + 1],
                                scale=SCALE,
                            )
                            p_tiles[(lqh, dc, m, br)] = p

            # ---------------- Phase 1: projections ----------------
            with ExitStack() as ph1:
                qinp = ph1.enter_context(tc.tile_pool(name="qinp", bufs=16))
                binp = ph1.enter_context(tc.tile_pool(name="binp", bufs=1))
                ppsum = ph1.enter_context(
                    tc.tile_pool(name="ppsum", bufs=2, space="PSUM")
                )

                # HAM warmup: keep the PE busy while the first input DMAs are
                # still in flight so the clock gate opens before real work.
                for i in range(8):
                    wp = ppsum.tile([128, 1024], f32, tag="ppsum", name="warmps")
                    nc.tensor.matmul(wp[:, 0:512], warm_sb[:, 0:128], warm_sb[:])

                # q chunks trickle in as individual DMAs, split across the
                # sync and scalar DMA queues so issue overhead parallelizes.
                xq_ch, wq_ch = [], []
                for k in range(8):
                    eng = nc.sync if k < 4 else nc.scalar
                    t = qinp.tile([128, LQ], bf16, tag="qinp", name=f"xq{k}")
                    eng.dma_start(out=t[:], in_=xqT[128 * k : 128 * k + 128, :])
                    xq_ch.append(t)
                    w = qinp.tile([128, HD], bf16, tag="qinpw", name=f"wq{k}")
                    eng.dma_start(out=w[:], in_=wqT[128 * k : 128 * k + 128, :])
                    wq_ch.append(w)

                # Bulk inputs/weights, batched 4-chunk DMAs split across the
                # gpsimd and scalar queues (halves so deps are finer-grained).
                bulk = {}
                for name, src, width, eng in (
                    ("xk", xkT, LKP, nc.gpsimd),
                    ("wk", wkT, HD, nc.gpsimd),
                    ("xr", xrT, LKP, nc.gpsimd),
                    ("wrk", wrkT, HD, nc.gpsimd),
                    ("xv", xvT, LKP, nc.scalar),
                    ("wv", wvT, HD, nc.scalar),
                    ("wrv", wrvT, HD, nc.scalar),
                ):
                    t = binp.tile([128, 8 * width], bf16, tag=f"b_{name}")
                    tv_ = t[:].rearrange("p (h k l) -> p h k l", h=2, k=4)
                    sv = src.rearrange("(h k p) l -> p h k l", h=2, k=4)
                    for hh in range(2):
                        eng.dma_start(out=tv_[:, hh, :, :], in_=sv[:, hh, :, :])
                    bulk[name] = t[:].rearrange("p (k l) -> p k l", k=8)

                # Output-projection weights last (needed late), one DMA.
                nc.scalar.dma_start(
                    out=wo_sb[:].rearrange("p (c d) -> p c d", c=4),
                    in_=woT.rearrange("(c p) d -> p c d", c=4),
                )

                # q projection: k-outer over dc pairs so matmuls start as
                # soon as the first chunks land (keeps the PE trickling).
                for pair in ((0, 1), (2, 3)):
                    pss = {}
                    for dcq in pair:
                        pss[dcq] = ppsum.tile(
                            [128, LQ], f32, tag="ppsum", name=f"psq{dcq}"
                        )
                    for k in range(8):
                        for dcq in pair:
                            for sl in (slice(0, 512), slice(512, 1024)):
                                nc.tensor.matmul(
                                    pss[dcq][:, sl],
                                    wq_ch[k][:, 128 * dcq : 128 * dcq + 128],
                                    xq_ch[k][:, sl],
                                    start=(k == 0),
                                    stop=(k == 7),
                                )
                    for dcq in pair:
                        nc.vector.tensor_scalar(
                            out=qT_sb[:, LQ * dcq : LQ * dcq + LQ],
                            in0=pss[dcq][:],
                            scalar1=bq_sb[:, dcq : dcq + 1],
                            scalar2=None,
                            op0=Add,
                        )

                # Transposed projections for k / rk (dc-outer; inputs are
                # fully resident by the time these run).
                for name, xn, wn, b_sb, out_sb in (
                    ("k", "xk", "wk", bk_sb, kT_sb),
                    ("rk", "xr", "wrk", brk_sb, rkT_sb),
                ):
                    xch, wch = bulk[xn], bulk[wn]
                    nsl = [slice(a, min(a + 512, LKP)) for a in range(0, LKP, 512)]
                    for dc in range(4):
                        ps = ppsum.tile([128, LKP], f32, tag="ppsum", name=f"ps{name}")
                        for k in range(8):
                            for sl in nsl:
                                nc.tensor.matmul(
                                    ps[:, sl],
                                    wch[:, k, 128 * dc : 128 * dc + 128],
                                    xch[:, k, sl],
                                    start=(k == 0),
                                    stop=(k == 7),
                                )
                        nc.vector.tensor_scalar(
                            out=out_sb[:, LKP * dc : LKP * dc + LKP],
                            in0=ps[:],
                            scalar1=b_sb[:, dc : dc + 1],
                            scalar2=None,
                            op0=Add,
                        )

                # Scores for the first lq half can start as soon as the
                # q/k/rk projections land.
                emit_scores(0)

                # Natural-orientation projections for v / rv.
                for name, xn, wn, b_sb, out4 in (
                    ("v", "xv", "wv", bv_sb, v4),
                    ("rv", "xr", "wrv", brv_sb, rv4),
                ):
                    xch, wch = bulk[xn], bulk[wn]
                    for m in range(NM):
                        ps = ppsum.tile([128, 512], f32, tag="ppsum", name=f"ps{name}")
                        for k in range(8):
                            nc.tensor.matmul(
                                ps[:],
                                xch[:, k, 128 * m : 128 * m + 128],
                                wch[:, k, :],
                                start=(k == 0),
                                stop=(k == 7),
                            )
                        nc.vector.tensor_tensor(
                            out=out4[:, m, :, :],
                            in0=ps[:].rearrange("p (h c) -> p h c", h=8, c=64),
                            in1=b_sb[:].rearrange("p (h c) -> p h c", h=8, c=64),
                            op=Add,
                        )

            emit_scores(1)

            # -------- Phase B: PV + denominators, normalize, out-proj ------
            with ExitStack() as ph2:
                work = ph2.enter_context(
                    tc.tile_pool(name="work", bufs=2, space="PSUM")
                )
                bcpool = ph2.enter_context(
                    tc.tile_pool(name="bcpool", bufs=2, space="PSUM")
                )
                rpool = ph2.enter_context(tc.tile_pool(name="rpool", bufs=4))
                tpool = ph2.enter_context(tc.tile_pool(name="tpool", bufs=4))
                ysb = ph2.enter_context(tc.tile_pool(name="ysb", bufs=4))

                def emit_outproj(lqh):
                    for ot in range(8):
                        acc = work.tile([128, 512], f32, tag="work", name=f"oy{ot}")
                        for dc in range(4):
                            nc.tensor.matmul(
                                acc[:],
                                wo_sb[:, 1024 * dc + 128 * ot : 1024 * dc + 128 * ot + 128],
                                xf_sb[
                                    :,
                                    1024 * dc + 512 * lqh : 1024 * dc + 512 * lqh + 512,
                                ],
                                start=(dc == 0),
                                stop=(dc == 3),
                            )
                        y = ysb.tile([128, 512], bf16, tag="ysb")
                        nc.vector.tensor_copy(out=y[:], in_=acc[:])
                        nc.sync.dma_start(
                            out=yT[
                                128 * ot : 128 * ot + 128, 512 * lqh : 512 * lqh + 512
                            ],
                            in_=y[:],
                        )

                for lqh in range(2):
                    for dc in range(4):
                        xv_ps = work.tile([128, 512], f32, tag="work", name="xv")
                        xr_ps = work.tile([128, 512], f32, tag="work", name="xr")
                        bcv_ps = bcpool.tile([128, 512], f32, tag="bcp", name="bcv")
                        bcr_ps = bcpool.tile([128, 512], f32, tag="bcp", name="bcr")
                        # Denominator-broadcast matmuls first: all-ones M=64
                        # weights put sum(p) on every one of the 64 partitions
                        # of each head's half, so their reciprocals run while
                        # the PV matmuls below keep the PE busy.
                        for m in range(NM):
                            st, sp = (m == 0), (m == NM - 1)
                            pv = p_tiles[(lqh, dc, m, 0)]
                            pr = p_tiles[(lqh, dc, m, 1)]
                            for bc_t, pt in ((bcv_ps, pv), (bcr_ps, pr)):
                                nc.tensor.matmul(
                                    bc_t[0:64, :], ones_bf[:], pt[:, 0:512],
                                    start=st, stop=sp,
                                )
                                nc.tensor.matmul(
                                    bc_t[64:128, :], ones_bf[:], pt[:, 512:1024],
                                    start=st, stop=sp,
                                )
                        for m in range(NM):
                            st, sp = (m == 0), (m == NM - 1)
                            pv = p_tiles[(lqh, dc, m, 0)]
                            pr = p_tiles[(lqh, dc, m, 1)]
                            for ps_t, vt, pt in ((xv_ps, v4, pv), (xr_ps, rv4, pr)):
                                nc.tensor.matmul(
                                    ps_t[0:64, :],
                                    vt[:, m, 2 * dc, :],
                                    pt[:, 0:512],
                                    start=st,
                                    stop=sp,
                                )
                                nc.tensor.matmul(
                                    ps_t[64:128, :],
                                    vt[:, m, 2 * dc + 1, :],
                                    pt[:, 512:1024],
                                    start=st,
                                    stop=sp,
                                )
                        rcv = rpool.tile([128, 512], f32, tag="rpool", name="rcv")
                        rcr = rpool.tile([128, 512], f32, tag="rpool", name="rcr")
                        nc.vector.reciprocal_approx_fast(out=rcv[:], in_=bcv_ps[:])
                        nc.vector.reciprocal_approx_fast(out=rcr[:], in_=bcr_ps[:])
                        if DEBUG_DUMP and lqh == 0 and dc == 0:
                            dd = tpool.tile([128, 512], f32, tag="dbgd")
                            nc.vector.tensor_copy(out=dd[:], in_=bcv_ps[:])
                            nc.sync.dma_start(out=dbg_den, in_=dd[:])
                            nc.sync.dma_start(out=dbg_rec, in_=rcv[:])
                            dx = tpool.tile([128, 512], f32, tag="dbgx")
                            nc.vector.tensor_copy(out=dx[:], in_=xv_ps[:])
                            nc.sync.dma_start(out=dbg_xv, in_=dx[:])
                        tv = tpool.tile([128, 512], bf16, tag="tpool", name="tv")
                        tr = tpool.tile([128, 512], bf16, tag="tpool", name="tr")
                        nc.vector.tensor_tensor(
                            out=tv[:], in0=xv_ps[:], in1=rcv[:], op=Mult
                        )
                        nc.vector.tensor_tensor(
                            out=tr[:], in0=xr_ps[:], in1=rcr[:], op=Mult
                        )
                        nc.vector.tensor_tensor(
                            out=xf_sb[
                                :, 1024 * dc + 512 * lqh : 1024 * dc + 512 * lqh + 512
                            ],
                            in0=tv[:],
                            in1=tr[:],
                            op=Add,
                        )
                emit_outproj(0)
                emit_outproj(1)

    nc.compile()
    return nc


def _get_program(lkp=LKP):
    if lkp not in _CACHE:
        _CACHE[lkp] = _build_program(lkp)
    return _CACHE[lkp]


def _bf16(arr):
    import ml_dtypes

    return np.ascontiguousarray(np.asarray(arr, dtype=np.float32).astype(ml_dtypes.bfloat16))


def _shard_inputs(inputs, lkp=LKP):
    q = np.ascontiguousarray(inputs["query"], dtype=np.float32)
    k = np.ascontiguousarray(inputs["key"], dtype=np.float32)
    v = np.ascontiguousarray(inputs["value"], dtype=np.float32)
    wr = np.ascontiguousarray(inputs["weak_rela"], dtype=np.float32)
    mask = np.asarray(inputs["mask"])

    in_maps = []
    for c in range(N_CORES):
        b, hh = divmod(c, 2)
        hsl = slice(HD * hh, HD * hh + HD)
        idx = np.nonzero(mask[b, 0])[0]
        nv = len(idx)
        assert nv <= lkp
        pidx = np.concatenate([idx, np.zeros(lkp - nv, dtype=idx.dtype)])
        bias = np.full(lkp, -1.0e9, np.float32)
        bias[:nv] = 0.0
        mb = np.ascontiguousarray(bias.reshape(lkp // 128, 128).T)
        kc, vc, wrc = k[b][pidx], v[b][pidx], wr[b][pidx]
        m = {
            "xqT": _bf16(q[b].T),
            "xkT": _bf16(kc.T),
            "xrT": _bf16(wrc.T),
            "xvT": _bf16(vc.T),
            "wqT": _bf16(np.asarray(inputs["Wq"])[hsl, :].T),
            "wkT": _bf16(np.asarray(inputs["Wk"])[hsl, :].T),
            "wrkT": _bf16(np.asarray(inputs["Wrk"])[hsl, :].T),
            "wvT": _bf16(np.asarray(inputs["Wv"])[hsl, :].T),
            "wrvT": _bf16(np.asarray(inputs["Wrv"])[hsl, :].T),
            "woT": _bf16(np.asarray(inputs["Wo"])[:, hsl].T),
            "bq_pc": np.asarray(inputs["bq"][hsl])
            .reshape(4, 128)
            .T.astype(np.float32),
            "bk_pc": np.asarray(inputs["bk"][hsl])
            .reshape(4, 128)
            .T.astype(np.float32),
            "brk_pc": np.asarray(inputs["brk"][hsl])
            .reshape(4, 128)
            .T.astype(np.float32),
            "bv_bc": np.broadcast_to(inputs["bv"][hsl], (128, HD)).astype(np.float32),
            "brv_bc": np.broadcast_to(inputs["brv"][hsl], (128, HD)).astype(
                np.float32
            ),
            "maskb": mb,
        }
        in_maps.append({k2: np.ascontiguousarray(v2) for k2, v2 in m.items()})
    return in_maps


def run_on_hw(inputs, trace=False, **kw):
    from concourse.bass_utils import run_bass_kernel_spmd

    mask = np.asarray(inputs["mask"])
    max_valid = max(int(mask[b, 0].sum()) for b in range(B))
    lkp = max(LKP, ((max_valid + 127) // 128) * 128)
    nc = _get_program(lkp)
    in_maps = _shard_inputs(inputs, lkp)
    res = run_bass_kernel_spmd(
        nc, in_maps, core_ids=list(range(N_CORES)), trace=trace, **kw
    )
    bo = np.asarray(inputs["bo"], dtype=np.float32)
    outs = []
    for b in range(B):
        yt = res.results[2 * b]["yT"].astype(np.float32) + res.results[
            2 * b + 1
        ]["yT"].astype(np.float32)
        outs.append(yt.T + bo)
    out = np.stack(outs).astype(np.float32)
    return out, res


def kernel(**inputs):
    out, _ = run_on_hw(inputs)
    return out


# revision 27
# speedup vs baseline: 1.1453x; 1.1453x over previous
"""Fused multi-head cross-attention with relation branch, sharded over 8 NeuronCores.

Sharding: data-parallel over batch (4) x tensor-parallel over head halves (2).
Core c handles batch c//2, heads [8*(c%2), 8*(c%2)+8). Each core computes its
partial output projection; the host sums the two partials per batch and adds bo.

Device data flow (per core), v2:
  - q/k/rk projections emitted transposed: qT/kT/rkT [512 local dims, L]
    (4 chunks of 128 dims = head pairs (2dc, 2dc+1) at partitions 0-63/64-127)
  - v/rv projections natural: per lk-chunk [128 lk, 8 heads x 64 dims].
  - scores sT[lk, lq] = kT.T @ qT per head; the two heads of a dim chunk run
    as one row-tiled concurrent pair (K=64 at array rows 0-63 / 64-127).
  - exp + mask + 1/sqrt(dk) fused into one ACT op per score tile.
  - PV: col-tiled concurrent pair per branch: head a -> psum rows 0:64
    (tile_position (0,0)), head b -> rows 64:128 ((0,64)); so xv/xr psum
    tiles land directly in the [2-head dims, lq] layout xf needs.
  - softmax denominators: 4-way col-tiled M=32 matmuls (all-ones weights)
    accumulate sum(p) into one psum tile (rows 0:32 = vis head a, 32:64 =
    vis head b, 64:96 = rel a, 96:128 = rel b); DVE reciprocal -> SBUF;
    gpsimd partition_broadcast expands each group row to 64 partitions;
    DVE combines xf = xv*rv + xr*rr. No DRAM round-trips.
  - output projection per lqh: 8 psum accumulators over 4 dim chunks,
    copies emitted as bf16, yT shipped bf16 (host sums partials in f32).
  - ~8 warmup matmuls on a memset tile at t~6.5us un-throttle the PE HAM
    clock gate before real data arrives.
"""

import math

import numpy as np

B, LQ, LK, D, H = 4, 1024, 1024, 1024, 16
DK = D // H
SCALE = 1.0 / math.sqrt(DK)
N_CORES = 8
HD = D // 2  # local dims per core (8 heads * 64)
# Keys are compacted host-side: only unmasked keys are shipped (padded to LKP
# with dummy rows whose mask bias is -1e9, so exp()=0 -> exact same math).
LKP = 640

DEBUG_DUMP = False

_CACHE = {}


def _build_program(lkp=LKP):
    import concourse.bacc as bacc
    import concourse.mybir as mybir
    import concourse.tile as tile

    LKP = lkp
    NM = LKP // 128

    f32 = mybir.dt.float32
    bf16 = mybir.dt.bfloat16
    Exp = mybir.ActivationFunctionType.Exp
    Add = mybir.AluOpType.add
    Mult = mybir.AluOpType.mult

    nc = bacc.Bacc(
        "TRN2",
        target_bir_lowering=False,
        debug=False,
        enable_asserts=False,
        num_devices=N_CORES,
    )

    # DRAM I/O (per-core shapes; host shards/pre-transposes/casts).
    xqT = nc.dram_tensor("xqT", [D, LQ], bf16, kind="ExternalInput").ap()
    xkT = nc.dram_tensor("xkT", [D, LKP], bf16, kind="ExternalInput").ap()
    xrT = nc.dram_tensor("xrT", [D, LKP], bf16, kind="ExternalInput").ap()
    xvT = nc.dram_tensor("xvT", [D, LKP], bf16, kind="ExternalInput").ap()
    wqT = nc.dram_tensor("wqT", [D, HD], bf16, kind="ExternalInput").ap()
    wkT = nc.dram_tensor("wkT", [D, HD], bf16, kind="ExternalInput").ap()
    wrkT = nc.dram_tensor("wrkT", [D, HD], bf16, kind="ExternalInput").ap()
    wvT = nc.dram_tensor("wvT", [D, HD], bf16, kind="ExternalInput").ap()
    wrvT = nc.dram_tensor("wrvT", [D, HD], bf16, kind="ExternalInput").ap()
    woT = nc.dram_tensor("woT", [HD, D], bf16, kind="ExternalInput").ap()
    bq_pc = nc.dram_tensor("bq_pc", [128, 4], f32, kind="ExternalInput").ap()
    bk_pc = nc.dram_tensor("bk_pc", [128, 4], f32, kind="ExternalInput").ap()
    brk_pc = nc.dram_tensor("brk_pc", [128, 4], f32, kind="ExternalInput").ap()
    bv_bc = nc.dram_tensor("bv_bc", [128, HD], f32, kind="ExternalInput").ap()
    brv_bc = nc.dram_tensor("brv_bc", [128, HD], f32, kind="ExternalInput").ap()
    maskb = nc.dram_tensor("maskb", [128, NM], f32, kind="ExternalInput").ap()
    yT = nc.dram_tensor("yT", [D, LQ], bf16, kind="ExternalOutput").ap()
    if DEBUG_DUMP:
        dbg_den = nc.dram_tensor("dbg_den", [128, 512], f32, kind="ExternalOutput").ap()
        dbg_rec = nc.dram_tensor("dbg_rec", [128, 512], f32, kind="ExternalOutput").ap()
        dbg_bcv = nc.dram_tensor("dbg_bcv", [128, 512], f32, kind="ExternalOutput").ap()
        dbg_xv = nc.dram_tensor("dbg_xv", [128, 512], f32, kind="ExternalOutput").ap()

    with tile.TileContext(nc) as tc:
        from contextlib import ExitStack

        with ExitStack() as ctx:
            # Persistent SBUF tensors.
            persist = ctx.enter_context(tc.tile_pool(name="persist", bufs=1))
            qT_sb = persist.tile([128, 4 * LQ], bf16, tag="qT")
            kT_sb = persist.tile([128, 4 * LKP], bf16, tag="kT")
            rkT_sb = persist.tile([128, 4 * LKP], bf16, tag="rkT")
            v_sb = persist.tile([128, NM * 8 * 64], bf16, tag="v")
            rv_sb = persist.tile([128, NM * 8 * 64], bf16, tag="rv")
            xf_sb = persist.tile([128, 4 * LQ], bf16, tag="xf")
            wo_sb = persist.tile([128, 4 * LQ], bf16, tag="wo")
            maskb_sb = persist.tile([128, NM], f32, tag="maskb")
            bq_sb = persist.tile([128, 4], f32, tag="bq")
            bk_sb = persist.tile([128, 4], f32, tag="bk")
            brk_sb = persist.tile([128, 4], f32, tag="brk")
            bv_sb = persist.tile([128, HD], f32, tag="bv")
            brv_sb = persist.tile([128, HD], f32, tag="brv")
            ones_bf = persist.tile([128, 64], bf16, tag="onesb")
            warm_sb = persist.tile([128, 512], bf16, tag="warm")

            # Memsets first: warmup matmuls depend only on these.
            nc.vector.memset(warm_sb[:], 0.125)
            nc.vector.memset(ones_bf[:], 1.0)

            # Small parameter DMAs on the gpsimd queue.
            nc.gpsimd.dma_start(out=maskb_sb[:], in_=maskb)
            nc.gpsimd.dma_start(out=bq_sb[:], in_=bq_pc)
            nc.gpsimd.dma_start(out=bk_sb[:], in_=bk_pc)
            nc.gpsimd.dma_start(out=brk_sb[:], in_=brk_pc)
            nc.gpsimd.dma_start(out=bv_sb[:], in_=bv_bc)
            nc.gpsimd.dma_start(out=brv_sb[:], in_=brv_bc)

            v4 = v_sb[:].rearrange("p (m h c) -> p m h c", m=NM, h=8, c=64)
            rv4 = rv_sb[:].rearrange("p (m h c) -> p m h c", m=NM, h=8, c=64)

            # Score/exp pools opened BEFORE the projection pools so their PSUM
            # banks are disjoint from the projection psum banks.
            spool = ctx.enter_context(tc.tile_pool(name="spool", bufs=2, space="PSUM"))
            ppool = ctx.enter_context(tc.tile_pool(name="ppool", bufs=24))

            p_tiles = {}

            def emit_scores(lqh):
                for dc in range(4):
                    qsl = slice(1024 * dc + 512 * lqh, 1024 * dc + 512 * lqh + 512)
                    for m in range(NM):
                        ksl = slice(LKP * dc + 128 * m, LKP * dc + 128 * m + 128)
                        for br, kt in ((0, kT_sb), (1, rkT_sb)):
                            s = spool.tile([128, 1024], f32, tag="spool", name="s")
                            nc.tensor.matmul(
                                s[:, 0:512], kt[0:64, ksl], qT_sb[0:64, qsl]
                            )
                            nc.tensor.matmul(
                                s[:, 512:1024], kt[64:128, ksl], qT_sb[64:128, qsl]
                            )
                            p = ppool.tile([128, 1024], bf16, tag="ppool", name="p")
                            nc.scalar.activation(
                                p[:],
                                s[:],
                                Exp,
                                bias=maskb_sb[:, m : m + 1],
                                scale=SCALE,
                            )
                            p_tiles[(lqh, dc, m, br)] = p

            # ---------------- Phase 1: projections ----------------
            with ExitStack() as ph1:
                qinp = ph1.enter_context(tc.tile_pool(name="qinp", bufs=16))
                binp = ph1.enter_context(tc.tile_pool(name="binp", bufs=1))
                ppsum = ph1.enter_context(
                    tc.tile_pool(name="ppsum", bufs=2, space="PSUM")
                )

                # HAM warmup: keep the PE busy while the first input DMAs are
                # still in flight so the clock gate opens before real work.
                for i in range(8):
                    wp = ppsum.tile([128, 1024], f32, tag="ppsum", name="warmps")
                    nc.tensor.matmul(wp[:, 0:512], warm_sb[:, 0:128], warm_sb[:])

                # q chunks trickle in as individual DMAs, split across the
                # sync and scalar DMA queues so issue overhead parallelizes.
                xq_ch, wq_ch = [], []
                for k in range(8):
                    eng = nc.sync if k < 4 else nc.scalar
                    t = qinp.tile([128, LQ], bf16, tag="qinp", name=f"xq{k}")
                    eng.dma_start(out=t[:], in_=xqT[128 * k : 128 * k + 128, :])
                    xq_ch.append(t)
                    w = qinp.tile([128, HD], bf16, tag="qinpw", name=f"wq{k}")
                    eng.dma_start(out=w[:], in_=wqT[128 * k : 128 * k + 128, :])
                    wq_ch.append(w)

                # Bulk inputs/weights, batched 4-chunk DMAs split across the
                # gpsimd and scalar queues. Halves of x and w interleave so
                # each projection's first chunks (x+w together) land early.
                bulk = {}
                views = {}
                for name, src, width, eng in (
                    ("xk", xkT, LKP, nc.gpsimd),
                    ("wk", wkT, HD, nc.gpsimd),
                    ("xr", xrT, LKP, nc.gpsimd),
                    ("wrk", wrkT, HD, nc.gpsimd),
                    ("xv", xvT, LKP, nc.scalar),
                    ("wv", wvT, HD, nc.scalar),
                    ("wrv", wrvT, HD, nc.scalar),
                ):
                    t = binp.tile([128, 8 * width], bf16, tag=f"b_{name}")
                    views[name] = (
                        t[:].rearrange("p (h k l) -> p h k l", h=2, k=4),
                        src.rearrange("(h k p) l -> p h k l", h=2, k=4),
                        eng,
                    )
                    bulk[name] = t[:].rearrange("p (k l) -> p k l", k=8)
                for pair_names in (("xk", "wk"), ("xr", "wrk"), ("xv", "wv")):
                    for hh in range(2):
                        for name in pair_names:
                            tv_, sv, eng = views[name]
                            eng.dma_start(out=tv_[:, hh, :, :], in_=sv[:, hh, :, :])
                for hh in range(2):
                    tv_, sv, eng = views["wrv"]
                    eng.dma_start(out=tv_[:, hh, :, :], in_=sv[:, hh, :, :])

                # Output-projection weights last (needed late), one DMA.
                nc.scalar.dma_start(
                    out=wo_sb[:].rearrange("p (c d) -> p c d", c=4),
                    in_=woT.rearrange("(c p) d -> p c d", c=4),
                )

                # q projection: k-outer over dc pairs so matmuls start as
                # soon as the first chunks land (keeps the PE trickling).
                for pair in ((0, 1), (2, 3)):
                    pss = {}
                    for dcq in pair:
                        pss[dcq] = ppsum.tile(
                            [128, LQ], f32, tag="ppsum", name=f"psq{dcq}"
                        )
                    for k in range(8):
                        for dcq in pair:
                            for sl in (slice(0, 512), slice(512, 1024)):
                                nc.tensor.matmul(
                                    pss[dcq][:, sl],
                                    wq_ch[k][:, 128 * dcq : 128 * dcq + 128],
                                    xq_ch[k][:, sl],
                                    start=(k == 0),
                                    stop=(k == 7),
                                )
                    for dcq in pair:
                        nc.vector.tensor_scalar(
                            out=qT_sb[:, LQ * dcq : LQ * dcq + LQ],
                            in0=pss[dcq][:],
                            scalar1=bq_sb[:, dcq : dcq + 1],
                            scalar2=None,
                            op0=Add,
                        )

                # Transposed projections for k / rk (dc-outer; inputs are
                # fully resident by the time these run).
                for name, xn, wn, b_sb, out_sb in (
                    ("k", "xk", "wk", bk_sb, kT_sb),
                    ("rk", "xr", "wrk", brk_sb, rkT_sb),
                ):
                    xch, wch = bulk[xn], bulk[wn]
                    nsl = [slice(a, min(a + 512, LKP)) for a in range(0, LKP, 512)]
                    for dc in range(4):
                        ps = ppsum.tile([128, LKP], f32, tag="ppsum", name=f"ps{name}")
                        for k in range(8):
                            for sl in nsl:
                                nc.tensor.matmul(
                                    ps[:, sl],
                                    wch[:, k, 128 * dc : 128 * dc + 128],
                                    xch[:, k, sl],
                                    start=(k == 0),
                                    stop=(k == 7),
                                )
                        nc.vector.tensor_scalar(
                            out=out_sb[:, LKP * dc : LKP * dc + LKP],
                            in0=ps[:],
                            scalar1=b_sb[:, dc : dc + 1],
                            scalar2=None,
                            op0=Add,
                        )

                # Scores for the first lq half can start as soon as the
                # q/k/rk projections land.
                emit_scores(0)

                # Natural-orientation projections for v / rv.
                for name, xn, wn, b_sb, out4 in (
                    ("v", "xv", "wv", bv_sb, v4),
                    ("rv", "xr", "wrv", brv_sb, rv4),
                ):
                    xch, wch = bulk[xn], bulk[wn]
                    for m in range(NM):
                        ps = ppsum.tile([128, 512], f32, tag="ppsum", name=f"ps{name}")
                        for k in range(8):
                            nc.tensor.matmul(
                                ps[:],
                                xch[:, k, 128 * m : 128 * m + 128],
                                wch[:, k, :],
                                start=(k == 0),
                                stop=(k == 7),
                            )
                        nc.vector.tensor_tensor(
                            out=out4[:, m, :, :],
                            in0=ps[:].rearrange("p (h c) -> p h c", h=8, c=64),
                            in1=b_sb[:].rearrange("p (h c) -> p h c", h=8, c=64),
                            op=Add,
                        )

            emit_scores(1)

            # -------- Phase B: PV + denominators, normalize, out-proj ------
            with ExitStack() as ph2:
                work = ph2.enter_context(
                    tc.tile_pool(name="work", bufs=2, space="PSUM")
                )
                bcpool = ph2.enter_context(
                    tc.tile_pool(name="bcpool", bufs=2, space="PSUM")
                )
                rpool = ph2.enter_context(tc.tile_pool(name="rpool", bufs=4))
                tpool = ph2.enter_context(tc.tile_pool(name="tpool", bufs=4))
                ysb = ph2.enter_context(tc.tile_pool(name="ysb", bufs=4))

                def emit_outproj(lqh):
                    for ot in range(8):
                        acc = work.tile([128, 512], f32, tag="work", name=f"oy{ot}")
                        for dc in range(4):
                            nc.tensor.matmul(
                                acc[:],
                                wo_sb[:, 1024 * dc + 128 * ot : 1024 * dc + 128 * ot + 128],
                                xf_sb[
                                    :,
                                    1024 * dc + 512 * lqh : 1024 * dc + 512 * lqh + 512,
                                ],
                                start=(dc == 0),
                                stop=(dc == 3),
                            )
                        y = ysb.tile([128, 512], bf16, tag="ysb")
                        nc.vector.tensor_copy(out=y[:], in_=acc[:])
                        nc.sync.dma_start(
                            out=yT[
                                128 * ot : 128 * ot + 128, 512 * lqh : 512 * lqh + 512
                            ],
                            in_=y[:],
                        )

                for lqh in range(2):
                    for dc in range(4):
                        if lqh == 1 and dc == 2:
                            emit_outproj(0)
                        xv_ps = work.tile([128, 512], f32, tag="work", name="xv")
                        xr_ps = work.tile([128, 512], f32, tag="work", name="xr")
                        bcv_ps = bcpool.tile([128, 512], f32, tag="bcp", name="bcv")
                        bcr_ps = bcpool.tile([128, 512], f32, tag="bcp", name="bcr")
                        # Denominator-broadcast matmuls first: all-ones M=64
                        # weights put sum(p) on every one of the 64 partitions
                        # of each head's half, so their reciprocals run while
                        # the PV matmuls below keep the PE busy.
                        for m in range(NM):
                            st, sp = (m == 0), (m == NM - 1)
                            pv = p_tiles[(lqh, dc, m, 0)]
                            pr = p_tiles[(lqh, dc, m, 1)]
                            for bc_t, pt in ((bcv_ps, pv), (bcr_ps, pr)):
                                nc.tensor.matmul(
                                    bc_t[0:64, :], ones_bf[:], pt[:, 0:512],
                                    start=st, stop=sp,
                                )
                                nc.tensor.matmul(
                                    bc_t[64:128, :], ones_bf[:], pt[:, 512:1024],
                                    start=st, stop=sp,
                                )
                        for m in range(NM):
                            st, sp = (m == 0), (m == NM - 1)
                            pv = p_tiles[(lqh, dc, m, 0)]
                            pr = p_tiles[(lqh, dc, m, 1)]
                            for ps_t, vt, pt in ((xv_ps, v4, pv), (xr_ps, rv4, pr)):
                                nc.tensor.matmul(
                                    ps_t[0:64, :],
                                    vt[:, m, 2 * dc, :],
                                    pt[:, 0:512],
                                    start=st,
                                    stop=sp,
                                )
                                nc.tensor.matmul(
                                    ps_t[64:128, :],
                                    vt[:, m, 2 * dc + 1, :],
                                    pt[:, 512:1024],
                                    start=st,
                                    stop=sp,
                                )
                        rcv = rpool.tile([128, 512], f32, tag="rpool", name="rcv")
                        rcr = rpool.tile([128, 512], f32, tag="rpool", name="rcr")
                        nc.vector.reciprocal_approx_fast(out=rcv[:], in_=bcv_ps[:])
                        nc.vector.reciprocal_approx_fast(out=rcr[:], in_=bcr_ps[:])
                        if DEBUG_DUMP and lqh == 0 and dc == 0:
                            dd = tpool.tile([128, 512], f32, tag="dbgd")
                            nc.vector.tensor_copy(out=dd[:], in_=bcv_ps[:])
                            nc.sync.dma_start(out=dbg_den, in_=dd[:])
                            nc.sync.dma_start(out=dbg_rec, in_=rcv[:])
                            dx = tpool.tile([128, 512], f32, tag="dbgx")
                            nc.vector.tensor_copy(out=dx[:], in_=xv_ps[:])
                            nc.sync.dma_start(out=dbg_xv, in_=dx[:])
                        tv = tpool.tile([128, 512], bf16, tag="tpool", name="tv")
                        tr = tpool.tile([128, 512], bf16, tag="tpool", name="tr")
                        nc.vector.tensor_tensor(
                            out=tv[:], in0=xv_ps[:], in1=rcv[:], op=Mult
                        )
                        nc.vector.tensor_tensor(
                            out=tr[:], in0=xr_ps[:], in1=rcr[:], op=Mult
                        )
                        nc.vector.tensor_tensor(
                            out=xf_sb[
                                :, 1024 * dc + 512 * lqh : 1024 * dc + 512 * lqh + 512
                            ],
                            in0=tv[:],
                            in1=tr[:],
                            op=Add,
                        )
                emit_outproj(1)

    nc.compile()
    return nc


def _get_program(lkp=LKP):
    if lkp not in _CACHE:
        _CACHE[lkp] = _build_program(lkp)
    return _CACHE[lkp]


def _bf16(arr):
    import ml_dtypes

    return np.ascontiguousarray(np.asarray(arr, dtype=np.float32).astype(ml_dtypes.bfloat16))


def _shard_inputs(inputs, lkp=LKP):
    q = np.ascontiguousarray(inputs["query"], dtype=np.float32)
    k = np.ascontiguousarray(inputs["key"], dtype=np.float32)
    v = np.ascontiguousarray(inputs["value"], dtype=np.float32)
    wr = np.ascontiguousarray(inputs["weak_rela"], dtype=np.float32)
    mask = np.asarray(inputs["mask"])

    in_maps = []
    for c in range(N_CORES):
        b, hh = divmod(c, 2)
        hsl = slice(HD * hh, HD * hh + HD)
        idx = np.nonzero(mask[b, 0])[0]
        nv = len(idx)
        assert nv <= lkp
        pidx = np.concatenate([idx, np.zeros(lkp - nv, dtype=idx.dtype)])
        bias = np.full(lkp, -1.0e9, np.float32)
        bias[:nv] = 0.0
        mb = np.ascontiguousarray(bias.reshape(lkp // 128, 128).T)
        kc, vc, wrc = k[b][pidx], v[b][pidx], wr[b][pidx]
        m = {
            "xqT": _bf16(q[b].T),
            "xkT": _bf16(kc.T),
            "xrT": _bf16(wrc.T),
            "xvT": _bf16(vc.T),
            "wqT": _bf16(np.asarray(inputs["Wq"])[hsl, :].T),
            "wkT": _bf16(np.asarray(inputs["Wk"])[hsl, :].T),
            "wrkT": _bf16(np.asarray(inputs["Wrk"])[hsl, :].T),
            "wvT": _bf16(np.asarray(inputs["Wv"])[hsl, :].T),
            "wrvT": _bf16(np.asarray(inputs["Wrv"])[hsl, :].T),
            "woT": _bf16(np.asarray(inputs["Wo"])[:, hsl].T),
            "bq_pc": np.asarray(inputs["bq"][hsl])
            .reshape(4, 128)
            .T.astype(np.float32),
            "bk_pc": np.asarray(inputs["bk"][hsl])
            .reshape(4, 128)
            .T.astype(np.float32),
            "brk_pc": np.asarray(inputs["brk"][hsl])
            .reshape(4, 128)
            .T.astype(np.float32),
            "bv_bc": np.broadcast_to(inputs["bv"][hsl], (128, HD)).astype(np.float32),
            "brv_bc": np.broadcast_to(inputs["brv"][hsl], (128, HD)).astype(
                np.float32
            ),
            "maskb": mb,
        }
        in_maps.append({k2: np.ascontiguousarray(v2) for k2, v2 in m.items()})
    return in_maps


def run_on_hw(inputs, trace=False, **kw):
    from concourse.bass_utils import run_bass_kernel_spmd

    mask = np.asarray(inputs["mask"])
    max_valid = max(int(mask[b, 0].sum()) for b in range(B))
    lkp = max(LKP, ((max_valid + 127) // 128) * 128)
    nc = _get_program(lkp)
    in_maps = _shard_inputs(inputs, lkp)
    res = run_bass_kernel_spmd(
        nc, in_maps, core_ids=list(range(N_CORES)), trace=trace, **kw
    )
    bo = np.asarray(inputs["bo"], dtype=np.float32)
    outs = []
    for b in range(B):
        yt = res.results[2 * b]["yT"].astype(np.float32) + res.results[
            2 * b + 1
        ]["yT"].astype(np.float32)
        outs.append(yt.T + bo)
    out = np.stack(outs).astype(np.float32)
    return out, res


def kernel(**inputs):
    out, _ = run_on_hw(inputs)
    return out
